# revision 6
# baseline (speedup 1.0000x reference)
"""Trainium2 Bass kernel for per-input-channel grouped 3x3 conv + bias +
round-quantize-dequantize + sum over input channels.

  reference: for each (batch, cin): y = conv3x3(x[cin], W[cin]) + b[cin]
             out = sum_cin round(S*y)/S          with S = 15/9 (RNE rounding)

Sharding: data-parallel over batch (B=8) across the 8 NeuronCores.

Per-core pipeline (28 blocks of 4 image rows = 448 pixels):
  1. DMA tap-expanded input rows: rhs_g [36, 448] fp32 (4 groups of 4 cins,
     each cin contributing 9 pre-shifted tap planes; zero padding baked in
     on the host).
  2. fp32 matmul per group: psum_g[128, 448] = lhsT_g[36,128].T @ rhs_g.
     lhsT is block-diagonal: partition (cin_l*9+tap) x column (cin_l*32+cout).
  3. Round pass: q16_g[128,448](int16) = convert_rne(S*psum_g + S*b) -- the
     fp32->int16 converter on ACT/DVE is exact round-nearest-even, which
     matches jnp.round.  Split across ScalarE (groups 0,1) and VectorE (2,3).
  4. int16 -> fp16 copy on GpSimdE (integer values |q|<2048 are exact fp16).
  5. fp16 reduce matmuls: opsum[32,448] += R[128,32].T @ q_g (R = 0/1 mask
     summing the 4 cins of each group; 4 groups accumulate in PSUM).
  6. Final: out[32,448] = opsum * (1/S) on VectorE, DMA to DRAM.
"""

import numpy as np

import concourse.bass as bass
import concourse.tile as tile
from concourse import bacc, mybir
from concourse.bass_utils import run_bass_kernel_spmd

F32 = mybir.dt.float32
F16 = mybir.dt.float16
I16 = mybir.dt.int16

# Problem shapes (hardcoded; harness calls kernel() with exactly these).
B, CIN, H, WID = 8, 16, 112, 112
COUT, KH, KW = 32, 3, 3
S = 15.0 / 9.0

N_CORES = 8
GROUPS = 4            # cin groups of 4
CIN_PER_G = CIN // GROUPS
K_CONV = CIN_PER_G * KH * KW        # 36
ROWS_PER_BLK = 4
NPIX = H * WID                      # 12544
NBLK = H // ROWS_PER_BLK            # 28
NFREE = ROWS_PER_BLK * WID          # 448

_CACHED = {}


def _build_program(loop_n=None):
    """Build the per-core program.  loop_n: if set, wrap the whole block loop
    in an on-device For_i repeat (bench-only; output is idempotent)."""
    from contextlib import ExitStack

    nc = bacc.Bacc(None, target_bir_lowering=False)

    xt_d = nc.declare_dram_parameter("xt", [NBLK, GROUPS * K_CONV, NFREE], F32,
                                     isOutput=False)
    wl_d = nc.declare_dram_parameter("wl", [K_CONV, GROUPS, 128], F32,
                                     isOutput=False)
    bias_d = nc.declare_dram_parameter("bias", [128, GROUPS], F32,
                                       isOutput=False)
    red_d = nc.declare_dram_parameter("red", [128, COUT], F16, isOutput=False)
    y_d = nc.declare_dram_parameter("y", [NBLK, COUT, NFREE], F32,
                                    isOutput=True)

    with tile.TileContext(nc) as tc:
        with (
            tc.tile_pool(name="consts", bufs=1) as consts,
            tc.tile_pool(name="rhs", bufs=8) as rhsp,
            tc.tile_pool(name="work", bufs=3) as work,
            tc.tile_pool(name="outp", bufs=3) as outp,
            tc.tile_pool(name="cps", bufs=3, space="PSUM") as cps,
            tc.tile_pool(name="ops", bufs=2, space="PSUM") as ops,
        ):
            wl_sb = consts.tile([K_CONV, GROUPS, 128], F32, tag="wl")
            bias_sb = consts.tile([128, GROUPS], F32, tag="bias")
            red_sb = consts.tile([128, COUT], F16, tag="red")
            nc.sync.dma_start(out=wl_sb, in_=wl_d[:, :, :])
            nc.sync.dma_start(out=bias_sb, in_=bias_d[:, :])
            nc.sync.dma_start(out=red_sb, in_=red_d[:, :])

            ctx = ExitStack()
            if loop_n is not None:
                ctx.enter_context(tc.For_i(0, loop_n, 1))

            for blk in range(NBLK):
                rhs = []
                for g in range(GROUPS):
                    r = rhsp.tile([K_CONV, NFREE], F32, tag=f"rhs{g}")
                    nc.sync.dma_start(
                        out=r,
                        in_=xt_d[blk, g * K_CONV:(g + 1) * K_CONV, :],
                    )
                    rhs.append(r)

                # Two PSUM tiles, two banks each (free dim padded to 512).
                pt01 = cps.tile([128, 2, 512], F32, tag="cps")
                pt23 = cps.tile([128, 2, 512], F32, tag="cps")
                for g in range(GROUPS):
                    pt = pt01 if g < 2 else pt23
                    nc.tensor.matmul(
                        out=pt[:, g % 2, :NFREE],
                        lhsT=wl_sb[:, g, :],
                        rhs=rhs[g],
                        start=True, stop=True,
                    )

                q16 = work.tile([128, GROUPS, NFREE], I16, tag="q16")
                # groups 0,1 on ScalarE: round(S*psum + S*b) via int16 convert
                for g in range(2):
                    nc.scalar.activation(
                        out=q16[:, g, :],
                        in_=pt01[:, g, :NFREE],
                        func=mybir.ActivationFunctionType.Identity,
                        scale=S,
                        bias=bias_sb[:, g:g + 1],
                    )
                # groups 2,3 on VectorE
                for g in range(2, 4):
                    nc.vector.tensor_scalar(
                        out=q16[:, g, :],
                        in0=pt23[:, g % 2, :NFREE],
                        scalar1=S,
                        scalar2=bias_sb[:, g:g + 1],
                        op0=mybir.AluOpType.mult,
                        op1=mybir.AluOpType.add,
                    )

                qf = work.tile([128, GROUPS, NFREE], F16, tag="qf")
                nc.gpsimd.tensor_copy(out=qf, in_=q16)

                op = ops.tile([COUT, NFREE], F32, tag="op")
                for g in range(GROUPS):
                    nc.tensor.matmul(
                        out=op,
                        lhsT=red_sb,
                        rhs=qf[:, g, :],
                        start=(g == 0), stop=(g == GROUPS - 1),
                    )

                osb = outp.tile([COUT, NFREE], F32, tag="osb")
                nc.vector.tensor_scalar_mul(out=osb, in0=op, scalar1=1.0 / S)
                nc.sync.dma_start(out=y_d[blk, :, :], in_=osb)

            ctx.close()

    nc.compile()
    return nc


def _prep_inputs(x, W, b):
    """Host-side preprocessing: tap expansion + block-major layout + consts."""
    x = np.ascontiguousarray(x, dtype=np.float32)
    Wt = np.asarray(W, dtype=np.float32)
    b = np.asarray(b, dtype=np.float32)

    # Tap-expanded input: xt[b, cin, tap, i, j] = x_pad[b, cin, i+dh, j+dw]
    xp = np.pad(x, ((0, 0), (0, 0), (1, 1), (1, 1)))
    sw = np.lib.stride_tricks.sliding_window_view(xp, (KH, KW), axis=(2, 3))
    # sw: [B, CIN, H, W, KH, KW] -> [B, CIN, KH*KW, H*W]
    xt = sw.transpose(0, 1, 4, 5, 2, 3).reshape(B, CIN * KH * KW, NPIX)
    # block-major: [B, NBLK, 144, NFREE]
    xt_b = np.ascontiguousarray(
        xt.reshape(B, CIN * KH * KW, NBLK, NFREE).transpose(0, 2, 1, 3))

    # Block-diagonal conv lhsT: wl[k, g, m]; k = cin_l*9 + tap, m = cin_l*32+cout
    wl = np.zeros((K_CONV, GROUPS, 128), dtype=np.float32)
    wf = Wt.reshape(CIN, COUT, KH * KW)
    for g in range(GROUPS):
        for cl in range(CIN_PER_G):
            c = g * CIN_PER_G + cl
            for t in range(KH * KW):
                wl[cl * 9 + t, g, cl * COUT:(cl + 1) * COUT] = wf[c, :, t]

    # Per-partition bias (pre-scaled by S): bias[cin_l*32+cout, g] = S*b[c, cout]
    bias = np.zeros((128, GROUPS), dtype=np.float32)
    for g in range(GROUPS):
        for cl in range(CIN_PER_G):
            c = g * CIN_PER_G + cl
            bias[cl * COUT:(cl + 1) * COUT, g] = S * b[c, :]

    # Reduce matrix: red[cin_l*32+cout, cout] = 1
    red = np.zeros((128, COUT), dtype=np.float16)
    for cl in range(CIN_PER_G):
        red[cl * COUT + np.arange(COUT), np.arange(COUT)] = 1.0

    return xt_b, wl, bias, red


def kernel(x, W, b):
    if "nc" not in _CACHED:
        _CACHED["nc"] = _build_program()
    nc = _CACHED["nc"]

    xt_b, wl, bias, red = _prep_inputs(x, W, b)
    in_maps = [
        {"xt": xt_b[i], "wl": wl, "bias": bias, "red": red}
        for i in range(N_CORES)
    ]
    res = run_bass_kernel_spmd(nc, in_maps, list(range(N_CORES)))

    out = np.empty((B, COUT, H, WID), dtype=np.float32)
    for i in range(N_CORES):
        # y: [NBLK, COUT, NFREE] -> [COUT, H, W]
        y = res.results[i]["y"]
        out[i] = y.transpose(1, 0, 2).reshape(COUT, H, WID)
    return out
